# revision 1
# baseline (speedup 1.0000x reference)
"""Causal self-attention Bass/Tile kernel for Trainium2, 8 NeuronCores (v5).

Sharding: batch (2) x head-groups (4 heads/core).  Host sums the 4 partial
output projections per batch.

v4: all matmul operands fp16 (PE 1 cycle/row vs fp32's 4).
v5: host pre-transposes x (kills the on-chip transpose phase), QKV
    projections pipelined against the x^T DMA, and the output projection
    is interleaved per-superblock into the attention loop so PE fills
    DVE/ACT stalls.  Output DMA'd as fp16; host sums partials in fp32.

Two-pass attention, no P transposes:
  pass 1 (stats): S[q,k] chunks in PSUM -> row max m (DVE), negated
  pass 2:        S^T[k,q] computed directly by a K=65 matmul where
                 row 64 of K^T is ones and row 64 of Q^T holds -m[q],
                 so PSUM already contains S^T - m.  ACT exp -> P^T fp16.
  diagonal causal masking via affine_select on P^T (zeros invalid k>q);
  AV matmul with [V | 1] fp16 appends the softmax denominator l as
  column 64; out rows scaled by 1/l during the PSUM->SBUF copy.
"""

import numpy as np

S = 2048
E = 1024
HPC = 4
D = 64
NCORES = 8
QB = S // 128     # 16 q-blocks
NSUP = S // 512   # 4 q-superblocks
KC = 8            # e chunks of 128
SCALE = 0.125     # 1/sqrt(64)

_CACHE = {}


def _build_nc():
    import concourse.bass as bass
    import concourse.mybir as mybir
    from concourse import tile

    f32 = mybir.dt.float32
    f32r = mybir.dt.float32r
    f16 = mybir.dt.float16
    X = mybir.AxisListType.X
    Exp = mybir.ActivationFunctionType.Exp

    nc = bass.Bass()

    xt_d = nc.declare_dram_parameter("xt", [E, S], f32, isOutput=False)
    wqk_d = nc.declare_dram_parameter("wqk", [E, HPC * 128], f32, isOutput=False)
    wv_d = nc.declare_dram_parameter("wv", [E, HPC * D], f32, isOutput=False)
    wo_d = nc.declare_dram_parameter("wo", [HPC * D, E], f16, isOutput=False)
    id_d = nc.declare_dram_parameter("ident", [128, 128], f16, isOutput=False)
    mask_d = nc.declare_dram_parameter("mask", [128, 128], f32, isOutput=False)
    out_d = nc.declare_dram_parameter("out", [S, E], f16, isOutput=True)

    with tile.TileContext(nc) as tc:
        with (
            tc.tile_pool(name="wpool", bufs=1) as wpool,
            tc.tile_pool(name="proj", bufs=1) as proj,
        ):
            wo = wpool.tile([128, 2, E], f16)
            ident = wpool.tile([128, 128], f16)
            mask = wpool.tile([128, 128], f32)

            # per-head QT/KT tiles [65, S]: rows 0-63 = projection (Q scaled),
            # row 64 of KT = ones, row 64 of QT = -m (written in phase C)
            qt = [proj.tile([65, S], f32, name=f"qt{h}", tag=f"qt{h}") for h in range(HPC)]
            kt = [proj.tile([65, S], f32, name=f"kt{h}", tag=f"kt{h}") for h in range(HPC)]
            qt16 = [proj.tile([64, S], f16, name=f"qs{h}", tag=f"qs{h}")
                    for h in range(HPC)]
            kt16 = [proj.tile([64, S], f16, name=f"ks{h}", tag=f"ks{h}")
                    for h in range(HPC)]
            vones = proj.tile([128, QB, HPC, D + 1], f16)
            attn = proj.tile([128, QB, HPC * D], f16)
            # one superblock's worth, reused each sup
            aot = proj.tile([128, 2, 512], f16)

            # ---- phase B: QKV projections (pipelined with x^T DMA) ----
            # wqk/wv/xT live only for phase B; their SBUF is reclaimed for
            # phase C's P^T buffers when this pool closes.
            with (
                tc.tile_pool(name="xtp", bufs=1) as xtp,
                tc.tile_pool(name="qkv", bufs=4, space="PSUM") as qkv,
            ):
                # fused Q|K weights: wqk[:, c, h, 0:64] = W_q, [.., 64:128] = W_k
                wqk = xtp.tile([128, KC, HPC, 128], f32)
                wv = xtp.tile([128, KC, 256], f32)
                xT = xtp.tile([128, KC, S], f32)
                nc.sync.dma_start(
                    wqk[:], wqk_d[:].rearrange("(c p) d -> p c d", p=128)
                    .rearrange("p c (h e) -> p c h e", h=HPC))
                nc.sync.dma_start(wv[:], wv_d[:].rearrange("(c p) d -> p c d", p=128))
                xt_r = xt_d[:].rearrange("(c p) s -> p c s", p=128)
                for sc in range(4):
                    nc.sync.dma_start(xT[:, :, 512 * sc:512 * (sc + 1)],
                                      xt_r[:, :, 512 * sc:512 * (sc + 1)])
                nc.sync.dma_start(ident[:], id_d[:])
                nc.sync.dma_start(mask[:], mask_d[:])
                nc.sync.dma_start(wo[:], wo_d[:].rearrange("(c p) e -> p c e", p=128))
                nc.vector.memset(vones[:, :, :, D:D + 1], 1.0)
                for h in range(HPC):
                    nc.gpsimd.memset(kt[h][64:65, :], 1.0)
                for sc in range(4):
                    for h in range(HPC):
                        pqk = qkv.tile([128, 512], f32, tag="qkv")
                        for c in range(KC):
                            nc.tensor.matmul(
                                pqk[:], wqk[:, c, h, :],
                                xT[:, c, 512 * sc:512 * (sc + 1)],
                                start=(c == 0), stop=(c == KC - 1),
                            )
                        nc.scalar.mul(qt[h][0:D, 512 * sc:512 * (sc + 1)],
                                      pqk[0:D, :], SCALE)
                        nc.scalar.mul(qt16[h][:, 512 * sc:512 * (sc + 1)],
                                      pqk[0:D, :], SCALE)
                        nc.vector.tensor_copy(kt[h][0:D, 512 * sc:512 * (sc + 1)],
                                              pqk[D:128, :])
                        nc.vector.tensor_copy(kt16[h][:, 512 * sc:512 * (sc + 1)],
                                              pqk[D:128, :])
                    for j in range(4 * sc, 4 * (sc + 1)):
                        pv = qkv.tile([128, 256], f32, tag="qkv")
                        for c in range(KC):
                            nc.tensor.matmul(
                                pv[:], xT[:, c, 128 * j:128 * (j + 1)], wv[:, c, :],
                                start=(c == 0), stop=(c == KC - 1),
                            )
                        nc.scalar.copy(
                            vones[:, j, :, 0:D],
                            pv[:].rearrange("p (h d) -> p h d", h=HPC),
                        )

            # ---- phase C: attention, with the output projection for each
            # finished superblock interleaved (phase D) ----
            with (
                tc.tile_pool(name="sc", bufs=2, space="PSUM") as scp,
                tc.tile_pool(name="st", bufs=2, space="PSUM") as stp,
                tc.tile_pool(name="axp", bufs=2, space="PSUM") as axp,
                tc.tile_pool(name="dps", bufs=2, space="PSUM") as dps,
                tc.tile_pool(name="pbuf", bufs=2) as pbuf,
                tc.tile_pool(name="stat", bufs=4) as stat,
                tc.tile_pool(name="osb", bufs=3) as osb,
            ):
                def emit_pass1(sup, h):
                    # row-max stats for the 4 sub-blocks -> negm4 (fp16)
                    negm4 = stat.tile([128, 4], f16, tag="n4")
                    for r in range(4):
                        i = 4 * sup + r
                        kn = 128 * (i + 1)
                        nch = (kn + 511) // 512
                        mparts = (stat.tile([128, 4], f32, tag="mp", name="mparts")
                                  if nch > 1 else None)
                        for c in range(nch):
                            n = min(512, kn - 512 * c)
                            sp = scp.tile([128, 512], f32, tag="sc")
                            nc.tensor.matmul(
                                sp[:, :n],
                                qt16[h][:, 128 * i:128 * (i + 1)],
                                kt16[h][:, 512 * c:512 * c + n],
                            )
                            if c == nch - 1:
                                # diagonal 128 cols: additive causal mask
                                nc.vector.tensor_add(
                                    sp[:, n - 128:n], sp[:, n - 128:n], mask[:])
                            if nch == 1:
                                nc.vector.reduce_max(negm4[:, r:r + 1], sp[:, :n],
                                                     axis=X, negate=True)
                            else:
                                nc.vector.reduce_max(mparts[:, c:c + 1], sp[:, :n],
                                                     axis=X)
                        if nch > 1:
                            nc.vector.reduce_max(negm4[:, r:r + 1], mparts[:, :nch],
                                                 axis=X, negate=True)
                    return negm4

                def emit_negm_pass2(sup, h, negm4):
                    # --- transpose -m into qt row 64 (single 512-wide copy) ---
                    nt = axp.tile([1, 512], f32, tag="ax", name="nt")
                    for r in range(4):
                        nc.tensor.matmul(nt[:, 128 * r:128 * (r + 1)],
                                         negm4[:, r:r + 1], ident[:])
                    nc.scalar.copy(qt[h][64:65, 512 * sup:512 * (sup + 1)], nt[0:1, :])
                    # --- pass 2: S^T - m, exp -> P^T fp16 ---
                    pt = pbuf.tile([128, QB, 512], f16, tag="pt")
                    jmax = 4 * (sup + 1)
                    for j in range(jmax):
                        # causal: q-sub-blocks left of j's diagonal are
                        # invalid; skip them (stale pt there is zeroed by
                        # the affine_select below and never read by AV)
                        q0 = 128 * max(0, j - 4 * sup)
                        st = stp.tile([128, 512], f32, tag="st")
                        nc.tensor.matmul(
                            st[:, q0:],
                            kt[h][0:D + 1, 128 * j:128 * (j + 1)],
                            qt[h][0:D + 1, 512 * sup + q0:512 * (sup + 1)],
                        )
                        nc.scalar.activation(pt[:, j, q0:], st[:, q0:], Exp)
                    # --- causal mask on the 4 diagonal blocks ---
                    for r in range(4):
                        sl = pt[:, 4 * sup + r, 128 * r:]
                        nc.gpsimd.affine_select(
                            sl, sl,
                            pattern=[[1, 512 - 128 * r]],
                            compare_op=mybir.AluOpType.is_ge,
                            fill=0.0,
                            base=0,
                            channel_multiplier=-1,
                        )
                    return pt

                def emit_av(sup, h, pt):
                    # --- AV + denominator + normalize ---
                    for r in range(4):
                        i = 4 * sup + r
                        av = axp.tile([128, D + 1], f32, tag="ax")
                        for j in range(i + 1):
                            nc.tensor.matmul(
                                av[:], pt[:, j, 128 * r:128 * (r + 1)],
                                vones[:, j, h, :],
                                start=(j == 0), stop=(j == i),
                            )
                        rl = stat.tile([128, 1], f32, tag="rl")
                        nc.vector.reciprocal(rl[:], av[:, D:D + 1])
                        nc.scalar.mul(attn[:, i, D * h:D * (h + 1)],
                                      av[:, 0:D], rl[:, 0:1])

                def emit_outproj(sup):
                    # attn^T for this superblock's 4 q-blocks + out proj + DMA
                    for db in range(2):
                        tp = dps.tile([128, 512], f32, tag="d")
                        for t in range(4):
                            i = 4 * sup + t
                            nc.tensor.matmul(
                                tp[:, 128 * t:128 * (t + 1)],
                                attn[:, i, 128 * db:128 * (db + 1)],
                                ident[:],
                            )
                        if db == 0:
                            nc.vector.tensor_copy(aot[:, db, :], tp[:])
                        else:
                            nc.scalar.copy(aot[:, db, :], tp[:])
                    for sb in range(4 * sup, 4 * (sup + 1)):
                        for ec in range(2):
                            po = dps.tile([128, 512], f32, tag="d")
                            for kb in range(2):
                                nc.tensor.matmul(
                                    po[:],
                                    aot[:, kb, 128 * (sb - 4 * sup):
                                        128 * (sb - 4 * sup + 1)],
                                    wo[:, kb, 512 * ec:512 * (ec + 1)],
                                    start=(kb == 0), stop=(kb == 1),
                                )
                            ob = osb.tile([128, 512], f16)
                            if ec == 0:
                                nc.scalar.copy(ob[:], po[:])
                            else:
                                nc.vector.tensor_copy(ob[:], po[:])
                            nc.sync.dma_start(
                                out_d[128 * sb:128 * (sb + 1),
                                      512 * ec:512 * (ec + 1)],
                                ob[:],
                            )

                # software-pipelined, depth 1: iteration N+1's pass1 issues
                # between pass2(N) and AV(N), so PE chews N+1's score matmuls
                # while ACT drains N's exp chain (AV's real gate), and DVE's
                # reductions for N+1 overlap AV(N) + negm(N+1).
                iters = [(sup, h) for sup in range(NSUP) for h in range(HPC)]
                negm = emit_pass1(*iters[0])
                pt = emit_negm_pass2(*iters[0], negm)
                for idx, (sup, h) in enumerate(iters):
                    nxt = iters[idx + 1] if idx + 1 < len(iters) else None
                    if nxt is not None:
                        negm = emit_pass1(*nxt)
                    emit_av(sup, h, pt)
                    if nxt is not None:
                        pt = emit_negm_pass2(*nxt, negm)
                    if h == HPC - 1:
                        emit_outproj(sup)

    _split_excess_waits(nc)
    return nc


def _split_excess_waits(nc, maxw=1):
    """walrus here accepts one sync-wait per instruction; Tile's tail drain
    aggregates several.  Hoist excess waits onto preceding same-engine nops."""
    import concourse.mybir as mybir

    f = nc.m.functions[0]
    for b in f.blocks:
        insts = b.instructions
        i = 0
        while i < len(insts):
            inst = insts[i]
            si = inst.sync_info
            if si and si.on_wait and len(si.on_wait) > maxw:
                waits = list(si.on_wait)
                si.on_wait = waits[-maxw:]
                pos = i
                for w in waits[:-maxw]:
                    nop = nc.engines[inst.engine].nop(
                        nofuse=True, hint="wait_split"
                    ).ins
                    for bb in f.blocks:
                        L = bb.instructions
                        for k in range(len(L) - 1, -1, -1):
                            if L[k] is nop:
                                L.pop(k)
                                break
                    nsi = nop.sync_info
                    if nsi is None:
                        nop.sync_info = mybir.SyncInfo(on_wait=[w], on_update=[])
                    else:
                        nsi.on_wait = [w]
                    insts.insert(pos, nop)
                    pos += 1
                    i += 1
            i += 1


def _get_nc():
    if "nc" not in _CACHE:
        _CACHE["nc"] = _build_nc()
    return _CACHE["nc"]


def _make_in_maps(x, W_q, W_k, W_v, W_o):
    ident = np.eye(128, dtype=np.float16)
    r = np.arange(128)
    mask_np = np.where(r[None, :] <= r[:, None], 0.0, -1.0e30).astype(np.float32)
    in_maps = []
    for c in range(NCORES):
        b, g = c // 4, c % 4
        cs = slice(256 * g, 256 * (g + 1))
        wq_s = W_q[:, cs].reshape(E, HPC, D)
        wk_s = W_k[:, cs].reshape(E, HPC, D)
        wqk_s = np.concatenate([wq_s, wk_s], axis=2).reshape(E, HPC * 128)
        in_maps.append({
            "xt": np.ascontiguousarray(x[b].T.astype(np.float32)),
            "wqk": np.ascontiguousarray(wqk_s).astype(np.float32),
            "mask": mask_np,
            "wv": np.ascontiguousarray(W_v[:, cs]).astype(np.float32),
            "wo": np.ascontiguousarray(W_o[cs, :]).astype(np.float16),
            "ident": ident,
        })
    return in_maps


def run_on_hw(x, W_q, W_k, W_v, W_o, trace=False):
    from concourse.bass_utils import run_bass_kernel_spmd

    nc = _get_nc()
    in_maps = _make_in_maps(x, W_q, W_k, W_v, W_o)
    res = run_bass_kernel_spmd(nc, in_maps, core_ids=list(range(NCORES)),
                               trace=trace)
    parts = [res.results[c]["out"].astype(np.float32) for c in range(NCORES)]
    out = np.stack([
        parts[0] + parts[1] + parts[2] + parts[3],
        parts[4] + parts[5] + parts[6] + parts[7],
    ]).astype(np.float32)
    return out, res


def kernel(x, W_q, W_k, W_v, W_o):
    x = np.asarray(x, dtype=np.float32)
    W_q = np.asarray(W_q, dtype=np.float32)
    W_k = np.asarray(W_k, dtype=np.float32)
    W_v = np.asarray(W_v, dtype=np.float32)
    W_o = np.asarray(W_o, dtype=np.float32)
    out, _ = run_on_hw(x, W_q, W_k, W_v, W_o, trace=False)
    return out



# revision 46
# speedup vs baseline: 1.0337x; 1.0337x over previous
"""Causal self-attention Bass/Tile kernel for Trainium2, 8 NeuronCores (v9).

Sharding: batch (2) x head-groups (4 heads/core).  Host sums the 4 partial
output projections per batch.

v9 (cost-model driven rewrite of v5, walrus-verifier clean):
  - All fp32 matmul operands are float32r end-to-end (DRAM params, SBUF
    tiles): 1 PE cycle/row (vs fp32's 4) when the moving dim is >=256.
    QKV projections and both attention score passes keep fp32-class
    precision at fp16 speed.
  - W_q is pre-scaled by 1/sqrt(D) on the host, so the PSUM->SBUF moves
    of Q^T/K^T are plain copies and there are no fp16 Q/K copies at all.
  - pass 1 fuses the causal mask INTO the row-max reduction: one
    tensor_tensor_reduce (S + mask, max) per score window with a chained
    per-partition init; no separate mask-adds or part combines.  The
    sign flip to -m happens in the nt eviction (mul -1).
  - pass 2 exps are batched in pairs over 2-bank PSUM tiles (one ACT
    instruction per two 512-wide S^T blocks) to halve ACT's per-
    instruction PSUM access overhead.
  - AV normalization: per-block divide by the PSUM ones-column (DVE
    tensor_scalar) or ACT-evict + Pool normalize_recip, per config.
  - GPSIMD (Pool) never touches PSUM (walrus rule): it owns the causal
    affine_selects on P^T and optional SBUF-side normalize work.
  - Fine-grained software pipeline: AV(N) / pass2(N+1) / pass1(N+2) are
    emitted unit-interleaved so the in-order engine queues always hold
    independent work behind any stalled instruction.

Two-pass attention, no P transposes (as v5):
  pass 1 (stats): S[q,k] windows in PSUM -> row-max via fused TTR chain
  pass 2:        S^T[k,q] computed by a K=65 matmul where row 64 of K^T
                 is ones and row 64 of Q^T holds -m[q]; ACT exp -> P^T f16
  diagonal causal masking via affine_select on P^T (zeros invalid k>q);
  AV matmul with [V | 1] f16 appends the softmax denominator as col 64.
"""

import numpy as np

S = 2048
E = 1024
HPC = 4
D = 64
NCORES = 8
QB = S // 128     # 16 q-blocks
NSUP = S // 512   # 4 q-superblocks
KC = 8            # e chunks of 128
SCALE = 0.125     # 1/sqrt(64), folded into W_q on the host

_CACHE = {}

# engine-assignment / buffering knobs (tuned against TimelineSim; KCFG env
# override is only used by the offline tuner, defaults are the tuned values)
_CFG = {
    "nt": "v",      # engine for nt -> qt row-64 negate-copy (v|s)
    "ob0": "s",     # engine for outproj eviction, ec 0 (v|s)
    "ob1": "s",     # engine for outproj eviction, ec 1 (v|s)
    "aot": "s",     # engine for attn^T eviction (v|s)
    "div": "v",     # AV normalize: v=DVE divide, p=ACT evict + Pool recip
    "qk0": "v",     # engine for q^T/k^T evictions (phase B), alternating
    "qk1": "s",
    "scb": 2,       # pass-1 score PSUM bufs (1 bank each)
    "stb": 2,       # pass-2 S^T PSUM bufs (2 banks each)
    "axb": 1,       # AV PSUM bufs
    "dpb": 1,       # outproj/nt PSUM bufs
    "ptb": 2,       # P^T SBUF bufs
    "order": "2o1a",  # drain round-robin order: a=av 2=pass2 1=pass1 o=outproj
}
import os as _os, json as _json
_CFG.update(_json.loads(_os.environ.get("KCFG", "{}")))


def _build_nc():
    import concourse.bass as bass
    import concourse.mybir as mybir
    from concourse import tile

    f32 = mybir.dt.float32
    f32r = mybir.dt.float32 if _CFG.get("nor") else mybir.dt.float32r
    f16 = mybir.dt.float16
    Exp = mybir.ActivationFunctionType.Exp
    X = mybir.AxisListType.X

    nc = bass.Bass()

    def _copy(code, dst, src):
        if code == "s":
            nc.scalar.copy(dst, src)
        else:
            nc.vector.tensor_copy(dst, src)

    xt_d = nc.declare_dram_parameter("xt", [E, S], f32r, isOutput=False)
    wqk_d = nc.declare_dram_parameter("wqk", [E, HPC * 128], f32r, isOutput=False)
    wv_d = nc.declare_dram_parameter("wv", [E, HPC * D], f32r, isOutput=False)
    wo_d = nc.declare_dram_parameter("wo", [HPC * D, E], f16, isOutput=False)
    id_d = nc.declare_dram_parameter("ident", [128, 128], f16, isOutput=False)
    m3_d = nc.declare_dram_parameter("m3", [128, 256], f16, isOutput=False)
    out_d = nc.declare_dram_parameter("out", [S, E], f16, isOutput=True)

    with tile.TileContext(nc) as tc:
        with (
            tc.tile_pool(name="wpool", bufs=1) as wpool,
            tc.tile_pool(name="proj", bufs=1) as proj,
        ):
            wo = wpool.tile([128, 2, E], f16)
            ident = wpool.tile([128, 128], f16)
            m3 = wpool.tile([128, 256], f16)

            # per-head QT/KT tiles [65, S] f32r: rows 0-63 = projection
            # (Q pre-scaled), row 64 of KT = ones, row 64 of QT = -m
            qt = [proj.tile([65, S], f32r, name=f"qt{h}", tag=f"qt{h}")
                  for h in range(HPC)]
            kt = [proj.tile([65, S], f32r, name=f"kt{h}", tag=f"kt{h}")
                  for h in range(HPC)]
            vones = proj.tile([128, QB, HPC, D + 1], f16)
            attn = proj.tile([128, QB, HPC * D], f16)
            # one superblock's worth, reused each sup
            aot = proj.tile([128, 2, 512], f16)

            # ---- phase B: QKV projections (pipelined with x^T DMA) ----
            with (
                tc.tile_pool(name="xtp", bufs=1) as xtp,
                tc.tile_pool(name="qkv", bufs=4, space="PSUM") as qkv,
            ):
                # fused Q|K weights: wqk[:, c, h, 0:64] = W_q*scale, [64:128] = W_k
                wqk = xtp.tile([128, KC, HPC, 128], f32r)
                wv = xtp.tile([128, KC, 256], f32r)
                xT = xtp.tile([128, KC, S], f32r)
                wqk_r = (wqk_d[:].rearrange("(c p) d -> p c d", p=128)
                         .rearrange("p c (h e) -> p c h e", h=HPC))
                xt_r = xt_d[:].rearrange("(c p) s -> p c s", p=128)
                # weights ride the ACT-issued DGE queue, x^T the SP queue:
                # two HWDGE rings issue in parallel; 256KB chunks so the
                # first tile's operands land fast
                for c in range(KC):
                    nc.sync.dma_start(xT[:, c, 0:512], xt_r[:, c, 0:512])
                    nc.scalar.dma_start(wqk[:, c], wqk_r[:, c])
                nc.scalar.dma_start(wv[:], wv_d[:].rearrange("(c p) d -> p c d", p=128))
                for sc in range(1, 4):
                    for c in range(KC):
                        nc.sync.dma_start(xT[:, c, 512 * sc:512 * (sc + 1)],
                                          xt_r[:, c, 512 * sc:512 * (sc + 1)])
                nc.scalar.dma_start(ident[:], id_d[:])
                nc.scalar.dma_start(m3[:], m3_d[:])
                nc.scalar.dma_start(wo[:], wo_d[:].rearrange("(c p) e -> p c e", p=128))
                nc.vector.memset(vones[:, :, :, D:D + 1], 1.0)
                for h in range(HPC):
                    nc.gpsimd.memset(kt[h][64:65, :].bitcast(f32), 1.0) if not _CFG.get("nor") else nc.gpsimd.memset(kt[h][64:65, :], 1.0)
                for sc in range(4):
                    for h in range(HPC):
                        pqk = qkv.tile([128, 512], f32, tag="qkv")
                        for c in range(KC):
                            nc.tensor.matmul(
                                pqk[:], wqk[:, c, h, :],
                                xT[:, c, 512 * sc:512 * (sc + 1)],
                                start=(c == 0), stop=(c == KC - 1),
                            )
                        # plain PSUM->SBUF evictions (W_q pre-scaled on host)
                        cols = slice(512 * sc, 512 * (sc + 1))
                        if h % 2 == 0:
                            _copy(_CFG["qk0"], qt[h][0:D, cols], pqk[0:D, :])
                            _copy(_CFG["qk1"], kt[h][0:D, cols], pqk[D:128, :])
                        else:
                            _copy(_CFG["qk1"], qt[h][0:D, cols], pqk[0:D, :])
                            _copy(_CFG["qk0"], kt[h][0:D, cols], pqk[D:128, :])
                    for j in range(4 * sc, 4 * (sc + 1)):
                        pv = qkv.tile([128, 256], f32, tag="qkv")
                        for c in range(KC):
                            nc.tensor.matmul(
                                pv[:], xT[:, c, 128 * j:128 * (j + 1)],
                                wv[:, c, :],
                                start=(c == 0), stop=(c == KC - 1),
                            )
                        nc.scalar.copy(
                            vones[:, j, :, 0:D],
                            pv[:].rearrange("p (h d) -> p h d", h=HPC),
                        )

            # ---- phase C: attention, with the output projection for each
            # finished superblock interleaved (phase D) ----
            with (
                tc.tile_pool(name="sc", bufs=_CFG["scb"], space="PSUM") as scp,
                tc.tile_pool(name="st", bufs=_CFG["stb"], space="PSUM") as stp,
                tc.tile_pool(name="axp", bufs=_CFG["axb"], space="PSUM") as axp,
                tc.tile_pool(name="dps", bufs=_CFG["dpb"], space="PSUM") as dps,
                tc.tile_pool(name="pbuf", bufs=_CFG["ptb"]) as pbuf,
                tc.tile_pool(name="stat", bufs=4) as stat,
                tc.tile_pool(name="osb", bufs=3) as osb,
            ):
                def gen_pass1(sup, h, slot):
                    # -row-max stats for the 4 sub-blocks -> m4 (f16, holds
                    # -m directly via negated reduces).  The causal mask is
                    # ADDED ON THE PE: a second accumulating matmul
                    # I^T @ m3 drops -6e4 onto invalid diag columns, so DVE
                    # only runs plain reduce_max per window plus one
                    # negating combine per multi-window row.
                    m4 = stat.tile([128, 4], f16, tag="n4")
                    slot["m4"] = m4
                    for r in range(4):
                        i = 4 * sup + r
                        kn = 128 * (i + 1)
                        # window list: (lo, width, mask_off | None); the
                        # masked window is last
                        wins = []
                        hi = kn - 512
                        while hi > 0:
                            if hi < 256:
                                wins.append((0, 256, None))
                                hi = 0
                            else:
                                lo = max(0, hi - 512)
                                wins.append((lo, hi - lo, None))
                                hi = lo
                        wins.reverse()
                        if i == 0:
                            wins.append((0, 256, 0))
                        else:
                            mw = min(512, kn)
                            wins.append((max(0, kn - mw), mw, mw - 128))
                        nch = len(wins)
                        parts = (stat.tile([128, 4], f32, tag="mp",
                                           name="parts") if nch > 1 else None)
                        for wi, (lo, w, moff) in enumerate(wins):
                            sp = scp.tile([128, 512], f32, tag="sc")
                            nc.tensor.matmul(
                                sp[:, :w],
                                qt[h][0:D, 128 * i:128 * (i + 1)],
                                kt[h][0:D, lo:lo + w],
                                start=True, stop=(moff is None),
                            )
                            if moff is not None:
                                mw2 = 256 if i == 0 else 128
                                nc.tensor.matmul(
                                    sp[:, moff:moff + mw2], ident[:],
                                    m3[:, 0:mw2],
                                    start=False, stop=True,
                                )
                            if nch == 1:
                                nc.vector.reduce_max(m4[:, r:r + 1],
                                                     sp[:, :w], axis=X,
                                                     negate=True)
                            else:
                                nc.vector.reduce_max(parts[:, wi:wi + 1],
                                                     sp[:, :w], axis=X)
                            yield
                        if nch > 1:
                            nc.vector.reduce_max(m4[:, r:r + 1],
                                                 parts[:, :nch], axis=X,
                                                 negate=True)

                def gen_negm_pass2(sup, h, m4, slot):
                    # --- transpose -m into qt row 64 ---
                    nt = dps.tile([1, 512], f32, tag="d", name="nt")
                    for r in range(4):
                        nc.tensor.matmul(nt[:, 128 * r:128 * (r + 1)],
                                         m4[:, r:r + 1], ident[:])
                    _copy(_CFG["nt"], qt[h][64:65, 512 * sup:512 * (sup + 1)],
                          nt[0:1, :])
                    yield
                    # --- pass 2: S^T - m in PSUM pairs, one exp per pair ---
                    pt = pbuf.tile([128, QB, 512], f16, tag="pt")
                    slot["pt"] = pt
                    jmax = 4 * (sup + 1)
                    # (j, q0, q0m): q0 = first causally-needed column;
                    # matmul starts at q0m (clamped so moving dim >= 256)
                    blocks = []
                    for j in range(jmax):
                        q0 = 128 * max(0, j - 4 * sup)
                        blocks.append((j, q0, min(q0, 256)))
                    # pair equal-shape neighbours into one 2-bank tile + 1 exp
                    bi = 0
                    while bi < len(blocks):
                        j0, q00, q0m0 = blocks[bi]
                        if (bi + 1 < len(blocks)
                                and blocks[bi + 1][1] == q00
                                and blocks[bi + 1][2] == q0m0):
                            j1 = blocks[bi + 1][0]
                            st = stp.tile([128, 2, 512], f32, tag="st")
                            for t, jj in enumerate((j0, j1)):
                                nc.tensor.matmul(
                                    st[:, t, q0m0:],
                                    kt[h][0:D + 1, 128 * jj:128 * (jj + 1)],
                                    qt[h][0:D + 1,
                                          512 * sup + q0m0:512 * (sup + 1)],
                                )
                            nc.scalar.activation(
                                pt[:, j0:j0 + 2, q00:],
                                st[:, :, q00:], Exp)
                            bi += 2
                        else:
                            st = stp.tile([128, 2, 512], f32, tag="st")
                            nc.tensor.matmul(
                                st[:, 0, q0m0:],
                                kt[h][0:D + 1, 128 * j0:128 * (j0 + 1)],
                                qt[h][0:D + 1,
                                      512 * sup + q0m0:512 * (sup + 1)],
                            )
                            nc.scalar.activation(pt[:, j0, q00:],
                                                 st[:, 0, q00:], Exp)
                            bi += 1
                        yield
                    # --- causal mask on the 4 diagonal blocks (Pool) ---
                    for r in range(4):
                        sl = pt[:, 4 * sup + r, 128 * r:]
                        nc.gpsimd.affine_select(
                            sl, sl,
                            pattern=[[1, 512 - 128 * r]],
                            compare_op=mybir.AluOpType.is_ge,
                            fill=0.0,
                            base=0,
                            channel_multiplier=-1,
                        )

                def gen_av(sup, h, pt):
                    # --- AV + denominator + normalize ---
                    for r in range(4):
                        i = 4 * sup + r
                        av = axp.tile([128, D + 1], f32, tag="ax")
                        for j in range(i + 1):
                            nc.tensor.matmul(
                                av[:], pt[:, j, 128 * r:128 * (r + 1)],
                                vones[:, j, h, :],
                                start=(j == 0), stop=(j == i),
                            )
                        if _CFG["div"] == "v":
                            rl = stat.tile([128, 1], f32, tag="rl")
                            nc.vector.reciprocal(rl[:], av[:, D:D + 1])
                            nc.scalar.mul(attn[:, i, D * h:D * (h + 1)],
                                          av[:, 0:D], rl[:, 0:1])
                        else:
                            # ACT evicts to SBUF, Pool divides (walrus: Pool
                            # cannot touch PSUM)
                            avs = stat.tile([128, D + 1], f32, tag="avs")
                            nc.scalar.copy(avs[:], av[:])
                            nc.gpsimd.normalize_recip(
                                attn[:, i, D * h:D * (h + 1)],
                                avs[:, 0:D], avs[:, D:D + 1],
                            )
                        yield

                def gen_outproj(sup):
                    # attn^T for this superblock's 4 q-blocks + out proj + DMA
                    for db in range(2):
                        tp = dps.tile([128, 512], f32, tag="d")
                        for t in range(4):
                            i = 4 * sup + t
                            nc.tensor.matmul(
                                tp[:, 128 * t:128 * (t + 1)],
                                attn[:, i, 128 * db:128 * (db + 1)],
                                ident[:],
                            )
                        _copy(_CFG["aot"], aot[:, db, :], tp[:])
                        yield
                    for sb in range(4 * sup, 4 * (sup + 1)):
                        for ec in range(2):
                            po = dps.tile([128, 512], f32, tag="d")
                            for kb in range(2):
                                nc.tensor.matmul(
                                    po[:],
                                    aot[:, kb, 128 * (sb - 4 * sup):
                                        128 * (sb - 4 * sup + 1)],
                                    wo[:, kb, 512 * ec:512 * (ec + 1)],
                                    start=(kb == 0), stop=(kb == 1),
                                )
                            ob = osb.tile([128, 512], f16)
                            _copy(_CFG["ob0"] if ec == 0 else _CFG["ob1"],
                                  ob[:], po[:])
                            nc.sync.dma_start(
                                out_d[128 * sb:128 * (sb + 1),
                                      512 * ec:512 * (ec + 1)],
                                ob[:],
                            )
                        yield

                def drain(*gens):
                    # round-robin one unit from each live stream so every
                    # engine's in-order queue gets work it can start on
                    live = [g for g in gens if g is not None]
                    while live:
                        nxt_live = []
                        for g in live:
                            try:
                                next(g)
                                nxt_live.append(g)
                            except StopIteration:
                                pass
                        live = nxt_live

                # software-pipelined, depth 2: iteration N runs AV while
                # N+1 runs pass2 and N+2 runs pass1, all unit-interleaved in
                # ONE drain so the strictly in-order PE queue always holds
                # independent matmuls behind any stalled pass-2 matmul
                # (which gate on ACT's exp freeing an st buffer).
                iters = [(sup, h) for sup in range(NSUP) for h in range(HPC)]
                slots = {}
                slots[0] = {}
                drain(gen_pass1(*iters[0], slots[0]))
                slots[1] = {}
                drain(gen_negm_pass2(*iters[0], slots[0]["m4"], slots[0]),
                      gen_pass1(*iters[1], slots[1]))
                for idx, (sup, h) in enumerate(iters):
                    av = gen_av(sup, h, slots[idx]["pt"])
                    p2 = p1 = None
                    if idx + 1 < len(iters):
                        p2 = gen_negm_pass2(*iters[idx + 1],
                                            slots[idx + 1]["m4"],
                                            slots[idx + 1])
                    if idx + 2 < len(iters):
                        slots[idx + 2] = {}
                        p1 = gen_pass1(*iters[idx + 2], slots[idx + 2])
                    op = gen_outproj(sup) if h == HPC - 1 else None
                    gmap = {"a": av, "2": p2, "1": p1, "o": op}
                    drain(*[gmap[c] for c in _CFG["order"]])
                    slots.pop(idx, None)

    _split_excess_waits(nc)
    return nc


def _split_excess_waits(nc, maxw=1):
    """walrus here accepts one sync-wait per instruction; Tile's tail drain
    aggregates several.  Hoist excess waits onto preceding same-engine nops."""
    import concourse.mybir as mybir

    f = nc.m.functions[0]
    for b in f.blocks:
        insts = b.instructions
        i = 0
        while i < len(insts):
            inst = insts[i]
            si = inst.sync_info
            if si and si.on_wait and len(si.on_wait) > maxw:
                waits = list(si.on_wait)
                si.on_wait = waits[-maxw:]
                pos = i
                for w in waits[:-maxw]:
                    nop = nc.engines[inst.engine].nop(
                        nofuse=True, hint="wait_split"
                    ).ins
                    for bb in f.blocks:
                        L = bb.instructions
                        for k in range(len(L) - 1, -1, -1):
                            if L[k] is nop:
                                L.pop(k)
                                break
                    nsi = nop.sync_info
                    if nsi is None:
                        nop.sync_info = mybir.SyncInfo(on_wait=[w], on_update=[])
                    else:
                        nsi.on_wait = [w]
                    insts.insert(pos, nop)
                    pos += 1
                    i += 1
            i += 1


def _get_nc():
    if "nc" not in _CACHE:
        _CACHE["nc"] = _build_nc()
    return _CACHE["nc"]


def _make_in_maps(x, W_q, W_k, W_v, W_o):
    ident = np.eye(128, dtype=np.float16)
    # pass-1 additive causal mask, applied BY THE PE (I^T @ m3 accumulated
    # onto the diagonal score block): cols 0-127 upper-triangular -6e4,
    # cols 128-255 all -6e4 (used only by q-block 0's extended window)
    cp = np.arange(128)[None, :]
    qq = np.arange(128)[:, None]
    m3 = np.zeros((128, 256), np.float16)
    m3[:, :128] = np.where(cp > qq, -6.0e4, 0.0).astype(np.float16)
    m3[:, 128:] = -6.0e4
    in_maps = []
    for c in range(NCORES):
        b, g = c // 4, c % 4
        cs = slice(256 * g, 256 * (g + 1))
        wq_s = (W_q[:, cs] * np.float32(SCALE)).reshape(E, HPC, D)
        wk_s = W_k[:, cs].reshape(E, HPC, D)
        wqk_s = np.concatenate([wq_s, wk_s], axis=2).reshape(E, HPC * 128)
        in_maps.append({
            "xt": np.ascontiguousarray(x[b].T.astype(np.float32)),
            "wqk": np.ascontiguousarray(wqk_s).astype(np.float32),
            "m3": m3,
            "wv": np.ascontiguousarray(W_v[:, cs]).astype(np.float32),
            "wo": np.ascontiguousarray(W_o[cs, :]).astype(np.float16),
            "ident": ident,
        })
    return in_maps


def run_on_hw(x, W_q, W_k, W_v, W_o, trace=False):
    from concourse.bass_utils import run_bass_kernel_spmd

    nc = _get_nc()
    in_maps = _make_in_maps(x, W_q, W_k, W_v, W_o)
    res = run_bass_kernel_spmd(nc, in_maps, core_ids=list(range(NCORES)),
                               trace=trace)
    parts = [res.results[c]["out"].astype(np.float32) for c in range(NCORES)]
    out = np.stack([
        parts[0] + parts[1] + parts[2] + parts[3],
        parts[4] + parts[5] + parts[6] + parts[7],
    ]).astype(np.float32)
    return out, res


def kernel(x, W_q, W_k, W_v, W_o):
    x = np.asarray(x, dtype=np.float32)
    W_q = np.asarray(W_q, dtype=np.float32)
    W_k = np.asarray(W_k, dtype=np.float32)
    W_v = np.asarray(W_v, dtype=np.float32)
    W_o = np.asarray(W_o, dtype=np.float32)
    out, _ = run_on_hw(x, W_q, W_k, W_v, W_o, trace=False)
    return out


# revision 49
# speedup vs baseline: 1.5336x; 1.4837x over previous
"""Causal self-attention Bass/Tile kernel for Trainium2, 8 NeuronCores (v9).

Sharding: batch (2) x head-groups (4 heads/core).  Host sums the 4 partial
output projections per batch.

v9 (cost-model driven rewrite of v5, walrus-verifier clean):
  - All fp32 matmul operands are float32r end-to-end (DRAM params, SBUF
    tiles): 1 PE cycle/row (vs fp32's 4) when the moving dim is >=256.
    QKV projections and both attention score passes keep fp32-class
    precision at fp16 speed.
  - W_q is pre-scaled by 1/sqrt(D) on the host, so the PSUM->SBUF moves
    of Q^T/K^T are plain copies and there are no fp16 Q/K copies at all.
  - pass 1 fuses the causal mask INTO the row-max reduction: one
    tensor_tensor_reduce (S + mask, max) per score window with a chained
    per-partition init; no separate mask-adds or part combines.  The
    sign flip to -m happens in the nt eviction (mul -1).
  - pass 2 exps are batched in pairs over 2-bank PSUM tiles (one ACT
    instruction per two 512-wide S^T blocks) to halve ACT's per-
    instruction PSUM access overhead.
  - AV normalization: per-block divide by the PSUM ones-column (DVE
    tensor_scalar) or ACT-evict + Pool normalize_recip, per config.
  - GPSIMD (Pool) never touches PSUM (walrus rule): it owns the causal
    affine_selects on P^T and optional SBUF-side normalize work.
  - Fine-grained software pipeline: AV(N) / pass2(N+1) / pass1(N+2) are
    emitted unit-interleaved so the in-order engine queues always hold
    independent work behind any stalled instruction.

Two-pass attention, no P transposes (as v5):
  pass 1 (stats): S[q,k] windows in PSUM -> row-max via fused TTR chain
  pass 2:        S^T[k,q] computed by a K=65 matmul where row 64 of K^T
                 is ones and row 64 of Q^T holds -m[q]; ACT exp -> P^T f16
  diagonal causal masking via affine_select on P^T (zeros invalid k>q);
  AV matmul with [V | 1] f16 appends the softmax denominator as col 64.
"""

import numpy as np

S = 2048
E = 1024
HPC = 4
D = 64
NCORES = 8
QB = S // 128     # 16 q-blocks
NSUP = S // 512   # 4 q-superblocks
KC = 8            # e chunks of 128
SCALE = 0.125     # 1/sqrt(64), folded into W_q on the host

_CACHE = {}

# engine-assignment / buffering knobs (tuned against TimelineSim; KCFG env
# override is only used by the offline tuner, defaults are the tuned values)
_CFG = {
    "nt": "v",      # engine for nt -> qt row-64 negate-copy (v|s)
    "ob0": "s",     # engine for outproj eviction, ec 0 (v|s)
    "ob1": "s",     # engine for outproj eviction, ec 1 (v|s)
    "aot": "s",     # engine for attn^T eviction (v|s)
    "div": "v",     # AV normalize: v=DVE divide, p=ACT evict + Pool recip
    "qk0": "v",     # engine for q^T/k^T evictions (phase B), alternating
    "qk1": "s",
    "scb": 2,       # pass-1 score PSUM bufs (1 bank each)
    "stb": 2,       # pass-2 S^T PSUM bufs (2 banks each)
    "axb": 1,       # AV PSUM bufs
    "dpb": 1,       # outproj/nt PSUM bufs
    "ptb": 2,       # P^T SBUF bufs
    "order": "2o1a",  # drain round-robin order: a=av 2=pass2 1=pass1 o=outproj
}
import os as _os, json as _json
_CFG.update(_json.loads(_os.environ.get("KCFG", "{}")))


def _build_nc():
    import concourse.bass as bass
    import concourse.mybir as mybir
    from concourse import tile

    f32 = mybir.dt.float32
    f16 = mybir.dt.float16
    Exp = mybir.ActivationFunctionType.Exp
    X = mybir.AxisListType.X

    nc = bass.Bass()

    def _copy(code, dst, src):
        if code == "s":
            nc.scalar.copy(dst, src)
        else:
            nc.vector.tensor_copy(dst, src)

    xh_d = nc.declare_dram_parameter("xh", [E, S], f16, isOutput=False)
    xl_d = nc.declare_dram_parameter("xl", [E, S], f16, isOutput=False)
    wqkh_d = nc.declare_dram_parameter("wqkh", [E, HPC * 128], f16, isOutput=False)
    wqkl_d = nc.declare_dram_parameter("wqkl", [E, HPC * 128], f16, isOutput=False)
    wv_d = nc.declare_dram_parameter("wv", [E, HPC * D], f16, isOutput=False)
    wo_d = nc.declare_dram_parameter("wo", [HPC * D, E], f16, isOutput=False)
    id_d = nc.declare_dram_parameter("ident", [128, 128], f16, isOutput=False)
    m3_d = nc.declare_dram_parameter("m3", [128, 128], f16, isOutput=False)
    out_d = nc.declare_dram_parameter("out", [S, E], f16, isOutput=True)

    with tile.TileContext(nc) as tc:
        with (
            tc.tile_pool(name="wpool", bufs=1) as wpool,
            tc.tile_pool(name="proj", bufs=1) as proj,
        ):
            wo = wpool.tile([128, 2, E], f16)
            ident = wpool.tile([128, 128], f16)
            m3 = wpool.tile([128, 128], f16)

            # per-head Q^T/K^T in f16 hi/lo split: q = qh + ql to ~22
            # mantissa bits.  hi tiles have row 64 (K^T: ones, Q^T: -m);
            # lo tiles are the 64 projection rows only.
            qh = [proj.tile([65, S], f16, name=f"qh{h}", tag=f"qh{h}")
                  for h in range(HPC)]
            ql = [proj.tile([64, S], f16, name=f"ql{h}", tag=f"ql{h}")
                  for h in range(HPC)]
            kh = [proj.tile([65, S], f16, name=f"kh{h}", tag=f"kh{h}")
                  for h in range(HPC)]
            kl = [proj.tile([64, S], f16, name=f"kl{h}", tag=f"kl{h}")
                  for h in range(HPC)]
            vones = proj.tile([128, QB, HPC, D + 1], f16)
            attn = proj.tile([128, QB, HPC * D], f16)
            # one superblock's worth, reused each sup
            aot = proj.tile([128, 2, 512], f16)

            # ---- phase B: QKV projections (pipelined with x^T DMA).
            # q = xh*Wh + xl*Wh + xh*Wl (f16 hi/lo, host-split operands);
            # V uses the single hi term (linear path, loose tolerance). ----
            with (
                tc.tile_pool(name="xtp", bufs=1) as xtp,
                tc.tile_pool(name="qkv", bufs=4, space="PSUM") as qkv,
            ):
                # fused Q|K weights: w*[:, c, h, 0:64] = W_q*scale, [64:128] = W_k
                wqkh = xtp.tile([128, KC, HPC, 128], f16)
                wqkl = xtp.tile([128, KC, HPC, 128], f16)
                wv = xtp.tile([128, KC, 256], f16)
                xTh = xtp.tile([128, KC, S], f16)
                xTl = xtp.tile([128, KC, S], f16)
                wqkh_r = (wqkh_d[:].rearrange("(c p) d -> p c d", p=128)
                          .rearrange("p c (h e) -> p c h e", h=HPC))
                wqkl_r = (wqkl_d[:].rearrange("(c p) d -> p c d", p=128)
                          .rearrange("p c (h e) -> p c h e", h=HPC))
                xh_r = xh_d[:].rearrange("(c p) s -> p c s", p=128)
                xl_r = xl_d[:].rearrange("(c p) s -> p c s", p=128)
                # weights ride the ACT-issued DGE queue, x^T the SP queue:
                # two HWDGE rings issue in parallel; 256KB chunks so the
                # first tile's operands land fast
                for c in range(KC):
                    nc.sync.dma_start(xTh[:, c, 0:512], xh_r[:, c, 0:512])
                    nc.sync.dma_start(xTl[:, c, 0:512], xl_r[:, c, 0:512])
                    nc.scalar.dma_start(wqkh[:, c], wqkh_r[:, c])
                    nc.scalar.dma_start(wqkl[:, c], wqkl_r[:, c])
                nc.scalar.dma_start(wv[:], wv_d[:].rearrange("(c p) d -> p c d", p=128))
                for sc in range(1, 4):
                    for c in range(KC):
                        nc.sync.dma_start(xTh[:, c, 512 * sc:512 * (sc + 1)],
                                          xh_r[:, c, 512 * sc:512 * (sc + 1)])
                        nc.sync.dma_start(xTl[:, c, 512 * sc:512 * (sc + 1)],
                                          xl_r[:, c, 512 * sc:512 * (sc + 1)])
                nc.scalar.dma_start(ident[:], id_d[:])
                nc.scalar.dma_start(m3[:], m3_d[:])
                nc.scalar.dma_start(wo[:], wo_d[:].rearrange("(c p) e -> p c e", p=128))
                nc.vector.memset(vones[:, :, :, D:D + 1], 1.0)
                for h in range(HPC):
                    nc.gpsimd.memset(kh[h][64:65, :], 1.0)
                for sc in range(4):
                    cols = slice(512 * sc, 512 * (sc + 1))
                    for h in range(HPC):
                        pqk = qkv.tile([128, 512], f32, tag="qkv")
                        for c in range(KC):
                            xs = xTh[:, c, cols]
                            nc.tensor.matmul(pqk[:], wqkh[:, c, h, :], xs,
                                             start=(c == 0), stop=False)
                        for c in range(KC):
                            nc.tensor.matmul(pqk[:], wqkl[:, c, h, :],
                                             xTh[:, c, cols],
                                             start=False, stop=False)
                        for c in range(KC):
                            nc.tensor.matmul(pqk[:], wqkh[:, c, h, :],
                                             xTl[:, c, cols],
                                             start=False, stop=(c == KC - 1))
                        # split evictions: hi = f16 round (ACT), lo = PSUM -
                        # hi residual (DVE)
                        nc.scalar.copy(qh[h][0:D, cols], pqk[0:D, :])
                        nc.vector.tensor_sub(ql[h][0:D, cols], pqk[0:D, :],
                                             qh[h][0:D, cols])
                        nc.scalar.copy(kh[h][0:D, cols], pqk[D:128, :])
                        nc.vector.tensor_sub(kl[h][0:D, cols], pqk[D:128, :],
                                             kh[h][0:D, cols])
                    for j in range(4 * sc, 4 * (sc + 1)):
                        pv = qkv.tile([128, 256], f32, tag="qkv")
                        for c in range(KC):
                            nc.tensor.matmul(
                                pv[:], xTh[:, c, 128 * j:128 * (j + 1)],
                                wv[:, c, :],
                                start=(c == 0), stop=(c == KC - 1),
                            )
                        nc.scalar.copy(
                            vones[:, j, :, 0:D],
                            pv[:].rearrange("p (h d) -> p h d", h=HPC),
                        )

            # ---- phase C: attention, with the output projection for each
            # finished superblock interleaved (phase D) ----
            with (
                tc.tile_pool(name="sc", bufs=_CFG["scb"], space="PSUM") as scp,
                tc.tile_pool(name="st", bufs=_CFG["stb"], space="PSUM") as stp,
                tc.tile_pool(name="axp", bufs=_CFG["axb"], space="PSUM") as axp,
                tc.tile_pool(name="dps", bufs=_CFG["dpb"], space="PSUM") as dps,
                tc.tile_pool(name="pbuf", bufs=_CFG["ptb"]) as pbuf,
                tc.tile_pool(name="stat", bufs=4) as stat,
                tc.tile_pool(name="osb", bufs=3) as osb,
            ):
                def gen_pass1(sup, h, slot):
                    # -row-max stats for the 4 sub-blocks -> m4 (f16, holds
                    # -m directly via negated reduces).  The causal mask is
                    # ADDED ON THE PE: a second accumulating matmul
                    # I^T @ m3 drops -6e4 onto invalid diag columns, so DVE
                    # only runs plain reduce_max per window plus one
                    # negating combine per multi-window row.
                    m4 = stat.tile([128, 4], f16, tag="n4")
                    slot["m4"] = m4
                    for r in range(4):
                        i = 4 * sup + r
                        kn = 128 * (i + 1)
                        # window list: (lo, width, mask_off | None); the
                        # masked window is last
                        wins = []
                        hi = kn - 512
                        while hi > 0:
                            lo = max(0, hi - 512)
                            wins.append((lo, hi - lo, None))
                            hi = lo
                        wins.reverse()
                        mw = min(512, kn)
                        wins.append((kn - mw, mw, mw - 128))
                        nch = len(wins)
                        parts = (stat.tile([128, 4], f32, tag="mp",
                                           name="parts") if nch > 1 else None)
                        for wi, (lo, w, moff) in enumerate(wins):
                            sp = scp.tile([128, 512], f32, tag="sc")
                            nc.tensor.matmul(
                                sp[:, :w],
                                qh[h][0:D, 128 * i:128 * (i + 1)],
                                kh[h][0:D, lo:lo + w],
                                start=True, stop=(moff is None),
                            )
                            if moff is not None:
                                nc.tensor.matmul(
                                    sp[:, moff:moff + 128], ident[:], m3[:],
                                    start=False, stop=True,
                                )
                            if nch == 1:
                                nc.vector.reduce_max(m4[:, r:r + 1],
                                                     sp[:, :w], axis=X,
                                                     negate=True)
                            else:
                                nc.vector.reduce_max(parts[:, wi:wi + 1],
                                                     sp[:, :w], axis=X)
                            yield
                        if nch > 1:
                            nc.vector.reduce_max(m4[:, r:r + 1],
                                                 parts[:, :nch], axis=X,
                                                 negate=True)

                def gen_negm_pass2(sup, h, m4, slot):
                    # --- transpose -m into qt row 64 ---
                    nt = dps.tile([1, 512], f32, tag="d", name="nt")
                    for r in range(4):
                        nc.tensor.matmul(nt[:, 128 * r:128 * (r + 1)],
                                         m4[:, r:r + 1], ident[:])
                    _copy(_CFG["nt"], qh[h][64:65, 512 * sup:512 * (sup + 1)],
                          nt[0:1, :])
                    yield
                    # --- pass 2: S^T - m in PSUM pairs, one exp per pair ---
                    pt = pbuf.tile([128, QB, 512], f16, tag="pt")
                    slot["pt"] = pt
                    jmax = 4 * (sup + 1)
                    # (j, q0): q0 = first causally-needed column (f16
                    # matmuls have no minimum-width penalty)
                    blocks = []
                    for j in range(jmax):
                        q0 = 128 * max(0, j - 4 * sup)
                        blocks.append((j, q0, q0))
                    # pair equal-shape neighbours into one 2-bank tile + 1 exp
                    bi = 0
                    while bi < len(blocks):
                        j0, q00, q0m0 = blocks[bi]
                        if (bi + 1 < len(blocks)
                                and blocks[bi + 1][1] == q00
                                and blocks[bi + 1][2] == q0m0):
                            j1 = blocks[bi + 1][0]
                            st = stp.tile([128, 2, 512], f32, tag="st")
                            for t, jj in enumerate((j0, j1)):
                                ks = slice(128 * jj, 128 * (jj + 1))
                                qs = slice(512 * sup + q0m0, 512 * (sup + 1))
                                nc.tensor.matmul(
                                    st[:, t, q0m0:], kh[h][0:D + 1, ks],
                                    qh[h][0:D + 1, qs],
                                    start=True, stop=False)
                                nc.tensor.matmul(
                                    st[:, t, q0m0:], kh[h][0:D, ks],
                                    ql[h][0:D, qs], start=False, stop=False)
                                nc.tensor.matmul(
                                    st[:, t, q0m0:], kl[h][0:D, ks],
                                    qh[h][0:D, qs], start=False, stop=True)
                            nc.scalar.activation(
                                pt[:, j0:j0 + 2, q00:],
                                st[:, :, q00:], Exp)
                            bi += 2
                        else:
                            st = stp.tile([128, 2, 512], f32, tag="st")
                            ks = slice(128 * j0, 128 * (j0 + 1))
                            qs = slice(512 * sup + q0m0, 512 * (sup + 1))
                            nc.tensor.matmul(
                                st[:, 0, q0m0:], kh[h][0:D + 1, ks],
                                qh[h][0:D + 1, qs], start=True, stop=False)
                            nc.tensor.matmul(
                                st[:, 0, q0m0:], kh[h][0:D, ks],
                                ql[h][0:D, qs], start=False, stop=False)
                            nc.tensor.matmul(
                                st[:, 0, q0m0:], kl[h][0:D, ks],
                                qh[h][0:D, qs], start=False, stop=True)
                            nc.scalar.activation(pt[:, j0, q00:],
                                                 st[:, 0, q00:], Exp)
                            bi += 1
                        yield
                    # --- causal mask on the 4 diagonal blocks (Pool) ---
                    for r in range(4):
                        sl = pt[:, 4 * sup + r, 128 * r:]
                        nc.gpsimd.affine_select(
                            sl, sl,
                            pattern=[[1, 512 - 128 * r]],
                            compare_op=mybir.AluOpType.is_ge,
                            fill=0.0,
                            base=0,
                            channel_multiplier=-1,
                        )

                def gen_av(sup, h, pt):
                    # --- AV + denominator + normalize ---
                    for r in range(4):
                        i = 4 * sup + r
                        av = axp.tile([128, D + 1], f32, tag="ax")
                        for j in range(i + 1):
                            nc.tensor.matmul(
                                av[:], pt[:, j, 128 * r:128 * (r + 1)],
                                vones[:, j, h, :],
                                start=(j == 0), stop=(j == i),
                            )
                        if _CFG["div"] == "v":
                            rl = stat.tile([128, 1], f32, tag="rl")
                            nc.vector.reciprocal(rl[:], av[:, D:D + 1])
                            nc.scalar.mul(attn[:, i, D * h:D * (h + 1)],
                                          av[:, 0:D], rl[:, 0:1])
                        else:
                            # ACT evicts to SBUF, Pool divides (walrus: Pool
                            # cannot touch PSUM)
                            avs = stat.tile([128, D + 1], f32, tag="avs")
                            nc.scalar.copy(avs[:], av[:])
                            nc.gpsimd.normalize_recip(
                                attn[:, i, D * h:D * (h + 1)],
                                avs[:, 0:D], avs[:, D:D + 1],
                            )
                        yield

                def gen_outproj(sup):
                    # attn^T for this superblock's 4 q-blocks + out proj + DMA
                    for db in range(2):
                        tp = dps.tile([128, 512], f32, tag="d")
                        for t in range(4):
                            i = 4 * sup + t
                            nc.tensor.matmul(
                                tp[:, 128 * t:128 * (t + 1)],
                                attn[:, i, 128 * db:128 * (db + 1)],
                                ident[:],
                            )
                        _copy(_CFG["aot"], aot[:, db, :], tp[:])
                        yield
                    for sb in range(4 * sup, 4 * (sup + 1)):
                        for ec in range(2):
                            po = dps.tile([128, 512], f32, tag="d")
                            for kb in range(2):
                                nc.tensor.matmul(
                                    po[:],
                                    aot[:, kb, 128 * (sb - 4 * sup):
                                        128 * (sb - 4 * sup + 1)],
                                    wo[:, kb, 512 * ec:512 * (ec + 1)],
                                    start=(kb == 0), stop=(kb == 1),
                                )
                            ob = osb.tile([128, 512], f16)
                            _copy(_CFG["ob0"] if ec == 0 else _CFG["ob1"],
                                  ob[:], po[:])
                            nc.sync.dma_start(
                                out_d[128 * sb:128 * (sb + 1),
                                      512 * ec:512 * (ec + 1)],
                                ob[:],
                            )
                        yield

                def drain(*gens):
                    # round-robin one unit from each live stream so every
                    # engine's in-order queue gets work it can start on
                    live = [g for g in gens if g is not None]
                    while live:
                        nxt_live = []
                        for g in live:
                            try:
                                next(g)
                                nxt_live.append(g)
                            except StopIteration:
                                pass
                        live = nxt_live

                # software-pipelined, depth 2: iteration N runs AV while
                # N+1 runs pass2 and N+2 runs pass1, all unit-interleaved in
                # ONE drain so the strictly in-order PE queue always holds
                # independent matmuls behind any stalled pass-2 matmul
                # (which gate on ACT's exp freeing an st buffer).
                iters = [(sup, h) for sup in range(NSUP) for h in range(HPC)]
                slots = {}
                slots[0] = {}
                drain(gen_pass1(*iters[0], slots[0]))
                slots[1] = {}
                drain(gen_negm_pass2(*iters[0], slots[0]["m4"], slots[0]),
                      gen_pass1(*iters[1], slots[1]))
                for idx, (sup, h) in enumerate(iters):
                    av = gen_av(sup, h, slots[idx]["pt"])
                    p2 = p1 = None
                    if idx + 1 < len(iters):
                        p2 = gen_negm_pass2(*iters[idx + 1],
                                            slots[idx + 1]["m4"],
                                            slots[idx + 1])
                    if idx + 2 < len(iters):
                        slots[idx + 2] = {}
                        p1 = gen_pass1(*iters[idx + 2], slots[idx + 2])
                    op = gen_outproj(sup) if h == HPC - 1 else None
                    gmap = {"a": av, "2": p2, "1": p1, "o": op}
                    drain(*[gmap[c] for c in _CFG["order"]])
                    slots.pop(idx, None)

    _split_excess_waits(nc)
    return nc


def _split_excess_waits(nc, maxw=1):
    """walrus here accepts one sync-wait per instruction; Tile's tail drain
    aggregates several.  Hoist excess waits onto preceding same-engine nops."""
    import concourse.mybir as mybir

    f = nc.m.functions[0]
    for b in f.blocks:
        insts = b.instructions
        i = 0
        while i < len(insts):
            inst = insts[i]
            si = inst.sync_info
            if si and si.on_wait and len(si.on_wait) > maxw:
                waits = list(si.on_wait)
                si.on_wait = waits[-maxw:]
                pos = i
                for w in waits[:-maxw]:
                    nop = nc.engines[inst.engine].nop(
                        nofuse=True, hint="wait_split"
                    ).ins
                    for bb in f.blocks:
                        L = bb.instructions
                        for k in range(len(L) - 1, -1, -1):
                            if L[k] is nop:
                                L.pop(k)
                                break
                    nsi = nop.sync_info
                    if nsi is None:
                        nop.sync_info = mybir.SyncInfo(on_wait=[w], on_update=[])
                    else:
                        nsi.on_wait = [w]
                    insts.insert(pos, nop)
                    pos += 1
                    i += 1
            i += 1


def _get_nc():
    if "nc" not in _CACHE:
        _CACHE["nc"] = _build_nc()
    return _CACHE["nc"]


def _make_in_maps(x, W_q, W_k, W_v, W_o):
    ident = np.eye(128, dtype=np.float16)
    # pass-1 additive causal mask, applied BY THE PE (I^T @ m3 accumulated
    # onto the diagonal score block): upper-triangular -6e4
    cp = np.arange(128)[None, :]
    qq = np.arange(128)[:, None]
    m3 = np.where(cp > qq, -6.0e4, 0.0).astype(np.float16)

    def split16(a):
        hi = a.astype(np.float16)
        lo = (a - hi.astype(np.float32)).astype(np.float16)
        return hi, lo

    xh = [None, None]
    xl = [None, None]
    for b in range(2):
        xh[b], xl[b] = split16(np.ascontiguousarray(x[b].T.astype(np.float32)))
    in_maps = []
    for c in range(NCORES):
        b, g = c // 4, c % 4
        cs = slice(256 * g, 256 * (g + 1))
        wq_s = (W_q[:, cs] * np.float32(SCALE)).reshape(E, HPC, D)
        wk_s = W_k[:, cs].reshape(E, HPC, D)
        wqk_s = np.ascontiguousarray(
            np.concatenate([wq_s, wk_s], axis=2).reshape(E, HPC * 128)
        ).astype(np.float32)
        wqkh, wqkl = split16(wqk_s)
        in_maps.append({
            "xh": xh[b],
            "xl": xl[b],
            "wqkh": wqkh,
            "wqkl": wqkl,
            "m3": m3,
            "wv": np.ascontiguousarray(W_v[:, cs]).astype(np.float16),
            "wo": np.ascontiguousarray(W_o[cs, :]).astype(np.float16),
            "ident": ident,
        })
    return in_maps


def run_on_hw(x, W_q, W_k, W_v, W_o, trace=False):
    from concourse.bass_utils import run_bass_kernel_spmd

    nc = _get_nc()
    in_maps = _make_in_maps(x, W_q, W_k, W_v, W_o)
    res = run_bass_kernel_spmd(nc, in_maps, core_ids=list(range(NCORES)),
                               trace=trace)
    parts = [res.results[c]["out"].astype(np.float32) for c in range(NCORES)]
    out = np.stack([
        parts[0] + parts[1] + parts[2] + parts[3],
        parts[4] + parts[5] + parts[6] + parts[7],
    ]).astype(np.float32)
    return out, res


def kernel(x, W_q, W_k, W_v, W_o):
    x = np.asarray(x, dtype=np.float32)
    W_q = np.asarray(W_q, dtype=np.float32)
    W_k = np.asarray(W_k, dtype=np.float32)
    W_v = np.asarray(W_v, dtype=np.float32)
    W_o = np.asarray(W_o, dtype=np.float32)
    out, _ = run_on_hw(x, W_q, W_k, W_v, W_o, trace=False)
    return out
